# revision 3
# baseline (speedup 1.0000x reference)
"""Modulated deformable conv (DFConv2d) Trainium2 Bass kernel, v5.
139044 ns on the timeline model (v4 baseline: 174538 ns).

Problem (hardcoded): x [4,256,64,64] f32; w_off [27,256,3,3]; b_off [27];
w_conv [256,256,3,3]; out [4,256,64,64].  K=3, pad=1, stride=1, dil=1.

Sharding: 8 cores = batch(4) x spatial-half(2).  Each core computes
out[b, :, s*32:(s+1)*32, :] (2048 output positions), in 4 chunks (cq)
of 512 positions.

v4's wall was SWDGE descriptor-gen on Pool: 144 indirect_dma_start
calls x 994ns fixed = 150us busy.  v5 replaces them with 5 dma_gather
calls per chunk (994ns + 0.34ns/descriptor) - dma_gather DOES work on
the deployed ucode provided (a) the int16 index tile is wrapped
[i%16, i//16] and replicated across the 8 Q7-core partition groups
(the executing DSP core is picked dynamically and reads its own
group), (b) num_idxs <= 1024 (the SWDGE descriptor ring is 16KB /
1024 entries - 1536 crashes the device).  Pool drops to ~43us and the
DMA bus becomes the wall: gather traffic is 2048 pos x 9 taps x 2KB
4-corner rows = 37.75MB @ 360GB/s = 105us, plus ~11us of input/output
traffic; everything else is scheduled to hide under it:
  * taps paired into 1024-idx gather calls (4x1024 + 1x512 per chunk,
    issued up-front): engine queues hold each call through its DMA
    transfer, so fewer/bigger calls let the in-order Pool queue reach
    the next chunk's calls ~1.5 chunks early - the bus never gaps.
  * pos-major offset conv: om^T[pos, 27] via swapped matmul operands
    (lhsT = xs window rows, rhs = wof).  walrus allows only one free
    dim on the moving operand, so the (row, col) window is split into
    two 64-position matmuls targeting partition halves of separate
    full-bank PSUM tiles (one pending accumulation group per bank).
    Offset bias is folded into the byt/bxt host tables (shared across
    chunks, + 8*cq added on device); mask-logit bias via a ones-column
    matmul.
  * wrapped-index bridge per chunk, all on PE/DVE: tfi [128p, 36]
    --PE transpose--> [36, 128] --8 sub-transposes into one PSUM
    tile--> [16, 8, 36] --1 strided DVE copy--> W16 [16, 288] f32
    --PE replicate matmul (tiled 16-identity, plain f32: float32r
    would round the indices!)--> [128, 288] --DVE int16 cast--> W128.
  * software pipeline: each body issues its 5 gathers, then the whole
    next-chunk head (conv+coords+bridge run in the dead time before
    the first transfer lands), then per-tap diag builds + combine.
  * w2 bf16 (same PE rate at free >= 256), bf16 output stores, merged
    single-DMA loads and one output DMA per chunk.
  * corner combine as in v4: four PSUM-accumulated diagonal matmuls
    per (tap, 128-pos group) fuse corner-blend + transpose on PE;
    diag(w) built 3/4 on DVE + 1/4 on ACT (both run within ~1.3us of
    the bus rate - the tail convergence of this chain sets the ~20us
    drain after the last transfer).
"""

import numpy as np

import concourse.bass as bass
import concourse.bacc as bacc
import concourse.tile as tile
from concourse import mybir
from concourse.bass_utils import run_bass_kernel_spmd

F32 = mybir.dt.float32
F32R = mybir.dt.float32r
BF16 = mybir.dt.bfloat16
I16 = mybir.dt.int16
AF = mybir.ActivationFunctionType
OP = mybir.AluOpType

B, C, H, W, O = 4, 256, 64, 64, 256
K2 = 9
POS = 2048                 # positions per core (32 output rows)
NCQ = 4                    # position chunks per core
CQP = POS // NCQ           # 512 positions per chunk
NT = 66                    # padded grid side (64 + 2 pad)
TROWS = NT * NT            # 4356 gather-table rows
MAGIC = 12582912.0         # 1.5*2^23 float-floor magic


def build_program(debug=False, hw_gather_walk=True, reps=1):
    nc = bacc.Bacc("TRN2", target_bir_lowering=False)

    xs_t = nc.dram_tensor("xs", (C, 34 * 66), BF16, kind="ExternalInput")
    wof_t = nc.dram_tensor("wof", (C, K2 * 27), BF16, kind="ExternalInput")
    bofm_t = nc.dram_tensor("bofm", (1, 27), BF16, kind="ExternalInput")
    w2_t = nc.dram_tensor("w2", (K2 * C, O), BF16, kind="ExternalInput")
    xq_t = nc.dram_tensor("xq", (TROWS, 4 * C), BF16, kind="ExternalInput")
    bytx_t = nc.dram_tensor("bytx", (128, 72), F32, kind="ExternalInput")
    idnb_t = nc.dram_tensor("idnb", (128, 128), BF16, kind="ExternalInput")
    idn128f_t = nc.dram_tensor("idn128f", (128, 128), F32, kind="ExternalInput")
    idn36f_t = nc.dram_tensor("idn36f", (36, 36), F32, kind="ExternalInput")
    i16rep_t = nc.dram_tensor("i16rep", (16, 128), F32, kind="ExternalInput")
    out_t = nc.dram_tensor("out", (O, POS), BF16, kind="ExternalOutput")

    with tile.TileContext(nc) as tc:
        with (
            tc.tile_pool(name="const", bufs=1) as constp,
            tc.tile_pool(name="coord", bufs=2) as coordp,
            tc.tile_pool(name="wrap", bufs=2) as wrapp,
            tc.tile_pool(name="gbuf", bufs=6) as gbufp,
            tc.tile_pool(name="diag", bufs=28) as diagp,
            tc.tile_pool(name="samp", bufs=3) as sampp,
            tc.tile_pool(name="outb", bufs=2) as outp,
            tc.tile_pool(name="ps_om", bufs=2, space="PSUM") as ps_om,
            tc.tile_pool(name="ps_br", bufs=1, space="PSUM") as ps_br,
            tc.tile_pool(name="ps_s", bufs=3, space="PSUM") as ps_s,
            tc.tile_pool(name="ps_out", bufs=1, space="PSUM") as ps_out,
        ):
            # ---- critical-path loads first: chunk-0 conv inputs ----
            xs_sb = []
            wof_sb = []
            for ct in range(2):
                t = constp.tile([128, 34 * 66], BF16, tag=f"xs{ct}", name=f"xs{ct}")
                # split the load: the first 10 rows unblock chunk 0's conv
                nc.sync.dma_start(
                    out=t[:].rearrange("p (r c) -> p r c", r=34)[:, 0:10, :],
                    in_=xs_t[ct * 128:(ct + 1) * 128, :].rearrange(
                        "p (r c) -> p r c", r=34)[:, 0:10, :])
                xs_sb.append(t)
            wof = constp.tile([128, 2, K2 * 27], BF16, tag="wof", name="wof")
            nc.sync.dma_start(
                out=wof[:],
                in_=wof_t[:].rearrange("(ct p) e -> p ct e", ct=2))
            wof_sb = [wof[:, 0, :], wof[:, 1, :]]
            bytx = constp.tile([128, 72], F32, tag="bytx", name="bytx")
            nc.sync.dma_start(out=bytx[:], in_=bytx_t[:])
            byt, bxt = bytx[:, 0:36], bytx[:, 36:72]
            ones1 = constp.tile([1, 128], BF16, tag="ones1", name="ones1")
            nc.vector.memset(ones1[:], 1.0)
            bofm = constp.tile([1, 27], BF16, tag="bofm", name="bofm")
            nc.sync.dma_start(out=bofm[:], in_=bofm_t[:])
            idn128f = constp.tile([128, 128], F32, tag="idn128f", name="idn128f")
            nc.sync.dma_start(out=idn128f[:], in_=idn128f_t[:])
            idn36f = constp.tile([36, 36], F32, tag="idn36f", name="idn36f")
            nc.sync.dma_start(out=idn36f[:], in_=idn36f_t[:])
            idnb = constp.tile([128, 128], BF16, tag="idnb", name="idnb")
            nc.sync.dma_start(out=idnb[:], in_=idnb_t[:])
            i16rep = constp.tile([16, 128], F32, tag="i16rep", name="i16rep")
            nc.sync.dma_start(out=i16rep[:], in_=i16rep_t[:])

            # single shared register for the constant gather count
            nidx_reg = nc.gpsimd.to_reg(CQP)
            nidx2_reg = nc.gpsimd.to_reg(2 * CQP)

            # ---- PE warm-up spins (p-state ramp) on a tiny zeroed tile ----
            wu = constp.tile([128, 64], BF16, tag="wu", name="wu")
            nc.vector.memset(wu[:], 0.0)
            wups = ps_br.tile([64, 64], F32, tag="br", name="wups")
            for i in range(8):
                nc.tensor.matmul(wups[:], lhsT=wu[:], rhs=wu[:],
                                 start=True, stop=True)

            # ---- bulk loads (needed later): xs rest, w2 ----
            for ct in range(2):
                nc.sync.dma_start(
                    out=xs_sb[ct][:].rearrange("p (r c) -> p r c", r=34)
                    [:, 10:34, :],
                    in_=xs_t[ct * 128:(ct + 1) * 128, :].rearrange(
                        "p (r c) -> p r c", r=34)[:, 10:34, :])
            w2 = constp.tile([128, 18, O], BF16, tag="w2", name="w2")
            nc.sync.dma_start(
                out=w2[:],
                in_=w2_t[:].rearrange("(kt p) e -> p kt e", kt=18))
            w2_sb = [w2[:, kt, :] for kt in range(18)]

            def head_stages(cq):
                    """Head pipeline for chunk cq as a list of closures; the
                    driver interleaves one stage per body tap so the in-order
                    engine queues never park on a long-latency head chain."""
                    state = {}

                    def sm(tag, dt=F32):
                        return coordp.tile([128, 36], dt, tag=tag, name=tag)

                    def v3(ap):
                        return ap.rearrange("p (k c4) -> p k c4", k=K2)

                    def conv_group(c4):
                        # walrus allows only one free dim on the moving
                        # operand: split the (row, col) window per row,
                        # writing 64-partition halves of the PSUM tile
                        for r in range(2):
                            # separate PSUM tiles per row-half: a bank holds
                            # only one pending accumulation group
                            # full-bank tile: two half-filled tiles must not
                            # share a PSUM bank (one pending group per bank)
                            pom = ps_om.tile([128, 512], F32, tag="pom",
                                             name="pom")[:, 0:27]
                            first = True
                            for ct in range(2):
                                for k in range(K2):
                                    ki, kj = k // 3, k % 3
                                    lhsT = (xs_sb[ct][:]
                                            .rearrange("p (r c) -> p r c",
                                                       r=34)
                                            [:, cq * 8 + c4 * 2 + r + ki,
                                             kj: kj + 64])
                                    nc.tensor.matmul(
                                        pom[r * 64:(r + 1) * 64, :],
                                        lhsT=lhsT,
                                        rhs=wof_sb[ct][:, k * 27:(k + 1) * 27],
                                        start=first,
                                        stop=False,
                                    )
                                    first = False
                            # mask-logit bias via ones-column (offset bias is
                            # folded into byt/bxt host tables)
                            nc.tensor.matmul(pom[r * 64:(r + 1) * 64, :],
                                             lhsT=ones1[:, 0:64],
                                             rhs=bofm[:],
                                             start=False, stop=True)
                            nc.vector.tensor_copy(
                                out=state["omt"][r * 64:(r + 1) * 64, c4, :],
                                in_=pom[r * 64:(r + 1) * 64, :])

                    def st0():
                        # ---- pos-major offset conv: omt[128, c4, 27] ----
                        state["omt"] = coordp.tile([128, 4, 27], F32,
                                                   tag="omt", name="omt")
                        conv_group(0)
                        conv_group(1)

                    def st1():
                        conv_group(2)
                        conv_group(3)

                    def st2():
                        omt = state["omt"]
                        # ys = off_y + byt + 8*cq  (byt shared across chunks)
                        ys = sm("ys")
                        nc.vector.scalar_tensor_tensor(
                            out=v3(ys[:]), in0=omt[:, :, 0:18:2].rearrange(
                                "p c4 k -> p k c4"),
                            scalar=float(8 * cq),
                            in1=v3(byt), op0=OP.add, op1=OP.add)
                        xs_ = sm("xs_")
                        nc.vector.tensor_tensor(
                            out=v3(xs_[:]), in0=omt[:, :, 1:18:2].rearrange(
                                "p c4 k -> p k c4"),
                            in1=v3(bxt), op=OP.add)
                        state["ys"], state["xs_"] = ys, xs_

                    def floorf(v, tagp):
                        r = sm(tagp + "_r")
                        nc.vector.tensor_scalar(out=r[:], in0=v[:],
                                                scalar1=MAGIC, scalar2=None,
                                                op0=OP.add)
                        nc.vector.tensor_scalar(out=r[:], in0=r[:],
                                                scalar1=MAGIC, scalar2=None,
                                                op0=OP.subtract)
                        corr = sm(tagp + "_c")
                        nc.vector.tensor_tensor(out=corr[:], in0=r[:],
                                                in1=v[:], op=OP.is_gt)
                        f = sm(tagp + "_f")
                        nc.vector.tensor_tensor(out=f[:], in0=r[:],
                                                in1=corr[:], op=OP.subtract)
                        frac = sm(tagp + "_fr")
                        nc.vector.tensor_tensor(out=frac[:], in0=v[:],
                                                in1=f[:], op=OP.subtract)
                        return f, frac

                    def st3():
                        # py = floor(y)+1 (pad baked)
                        state["py"], state["ly"] = floorf(state["ys"], "fy")
                        state["px"], state["lx"] = floorf(state["xs_"], "fx")

                    def st4():
                        py, px = state["py"], state["px"]
                        pyc = sm("pyc")
                        nc.vector.tensor_scalar(out=pyc[:], in0=py[:],
                                                scalar1=0.0, scalar2=64.0,
                                                op0=OP.max, op1=OP.min)
                        pxc = sm("pxc")
                        nc.vector.tensor_scalar(out=pxc[:], in0=px[:],
                                                scalar1=0.0, scalar2=64.0,
                                                op0=OP.max, op1=OP.min)
                        vy = sm("vy")
                        nc.vector.tensor_tensor(out=vy[:], in0=pyc[:],
                                                in1=py[:], op=OP.is_equal)
                        vx = sm("vx")
                        nc.vector.tensor_tensor(out=vx[:], in0=pxc[:],
                                                in1=px[:], op=OP.is_equal)
                        vv = sm("vv")
                        nc.vector.tensor_tensor(out=vv[:], in0=vy[:],
                                                in1=vx[:], op=OP.mult)
                        # mask = sigmoid(logits) * validity
                        mk = sm("mk")
                        nc.scalar.activation(
                            out=v3(mk[:]),
                            in_=state["omt"][:, :, 18:27].rearrange(
                                "p c4 k -> p k c4"),
                            func=AF.Sigmoid)
                        mv = sm("mv")
                        nc.vector.tensor_tensor(out=mv[:], in0=mk[:],
                                                in1=vv[:], op=OP.mult)
                        state.update(pyc=pyc, pxc=pxc, mv=mv)

                    def st5():
                        ly, lx, mv = state["ly"], state["lx"], state["mv"]
                        # corner weights: cw[a][b] = wy_a * wx_b * mv
                        wx1m = sm("wx1m")
                        nc.vector.tensor_tensor(out=wx1m[:], in0=lx[:],
                                                in1=mv[:], op=OP.mult)
                        wx0m = sm("wx0m")
                        nc.vector.tensor_tensor(out=wx0m[:], in0=mv[:],
                                                in1=wx1m[:], op=OP.subtract)
                        wy0 = sm("wy0")
                        nc.vector.tensor_scalar(out=wy0[:], in0=ly[:],
                                                scalar1=-1.0, scalar2=1.0,
                                                op0=OP.mult, op1=OP.add)
                        cw = {}
                        for (a, wya) in ((0, wy0), (1, ly)):
                            for (b, wxb) in ((0, wx0m), (1, wx1m)):
                                t = sm(f"cw{a}{b}")
                                nc.vector.tensor_tensor(out=t[:], in0=wya[:],
                                                        in1=wxb[:], op=OP.mult)
                                cw[(a, b)] = t
                        state["cw"] = cw

                    def st6():
                        # ---- gather index: row = pyc*66 + pxc ----
                        tfi = sm("tfi")
                        nc.vector.tensor_scalar(out=tfi[:], in0=state["pyc"][:],
                                                scalar1=float(NT), scalar2=None,
                                                op0=OP.mult)
                        nc.vector.tensor_tensor(out=tfi[:], in0=tfi[:],
                                                in1=state["pxc"][:], op=OP.add)
                        # bridge part 1: T1[col, p] = tfi[p, col]
                        pt1 = ps_br.tile([36, 128], F32, tag="br", name="pt1")
                        nc.tensor.transpose(out=pt1[:], in_=tfi[:],
                                            identity=idn128f[:])
                        t1s = wrapp.tile([36, 128], F32, tag="t1s", name="t1s")
                        nc.vector.tensor_copy(out=t1s[:], in_=pt1[:])
                        state["t1s"] = t1s

                    def st7():
                        # bridge part 2: 8 sub-transposes into one PSUM tile,
                        # then a single strided int16 glue copy.
                        # W16[q, (k*4+c4)*8 + s] = tfi[16s+q, k*4+c4]
                        t1s = state["t1s"]
                        pc8 = ps_br.tile([16, 8, 36], F32, tag="br",
                                         name="pc8")
                        for s in range(8):
                            nc.tensor.transpose(
                                out=pc8[:, s, :],
                                in_=t1s[:, 16 * s:16 * s + 16],
                                identity=idn36f[:])
                        w16 = wrapp.tile([16, 288], F32, tag="w16", name="w16")
                        nc.vector.tensor_copy(
                            out=w16[:].rearrange("q (kc s) -> q s kc", s=8),
                            in_=pc8[:])
                        state["w16"] = w16

                    def st8():
                        # replicate across the 8 Q7-core partition groups via
                        # a PE matmul with a tiled identity (out[16g+q, e] =
                        # w16[q, e]), then an int16 cast copy
                        prep = ps_br.tile([128, 288], F32,
                                          tag="br", name="prep")
                        nc.tensor.matmul(prep[:],
                                         lhsT=i16rep[:],
                                         rhs=state["w16"][:],
                                         start=True, stop=True)
                        w128 = wrapp.tile([128, 288], I16, tag="w128",
                                          name="w128")
                        nc.vector.tensor_copy(out=w128[:], in_=prep[:])
                        state["w128"] = w128

                    return [st0, st1, st2, st3, st4, st5, st6, st7, st8], state

            CWO = [(0, 0), (0, 1), (1, 0), (1, 1)]  # corner order in xq row

            def body(cq, cw, w128, nstages=None):
                    # gathers issued up-front, taps paired into 1024-idx
                    # calls (4x1024 + 1x512): fewer Pool-queue slots per
                    # chunk, so the in-order engine queue (which holds each
                    # call through its DMA transfer) reaches the next chunk's
                    # calls ~1.5 chunks early and the bus never gaps
                    gs = []
                    for kp in range(4):
                        g = gbufp.tile([128, 8, 4 * C], BF16, tag="g2",
                                       name="g2")
                        nc.gpsimd.dma_gather(
                            out_ap=g[:], in_ap=xq_t[:],
                            idxs_ap=w128[:, kp * 64:(kp + 1) * 64],
                            num_idxs=2 * CQP, num_idxs_reg=nidx2_reg,
                            elem_size=4 * C,
                        )
                        gs.append(g)
                    g1 = gbufp.tile([128, 4, 4 * C], BF16, tag="g1", name="g1")
                    nc.gpsimd.dma_gather(
                        out_ap=g1[:], in_ap=xq_t[:],
                        idxs_ap=w128[:, 8 * 32:9 * 32],
                        num_idxs=CQP, num_idxs_reg=nidx_reg, elem_size=4 * C,
                    )
                    gs.append(g1)

                    # big-matmul PSUM accumulators, fed per tap
                    po = [ps_out.tile([128, CQP], F32, tag=f"po{m}",
                                      name=f"po{m}") for m in range(2)]
                    # run the entire next-chunk head now: PE/DVE are idle
                    # until the first gather transfer lands
                    if nstages is not None:
                        for stg in nstages:
                            stg()
                    for k in range(K2):
                        g = gs[k // 2][:, (k % 2) * 4:(k % 2) * 4 + 4, :] \
                            if k < 8 else gs[4][:]
                        dg = []
                        for c4 in range(4):
                            col = k * 4 + c4
                            ds = []
                            for j in range(4):
                                d = diagp.tile([128, 128], BF16, tag="dg",
                                               name="dg")
                                scal = cw[CWO[j]][:, col:col + 1]
                                if j == 3:
                                    nc.scalar.activation(
                                        out=d[:], in_=idnb[:], func=AF.Copy,
                                        scale=scal)
                                else:
                                    nc.vector.scalar_tensor_tensor(
                                        out=d[:], in0=idnb[:], scalar=scal,
                                        in1=idnb[:], op0=OP.mult,
                                        op1=OP.bypass)
                                ds.append(d)
                            dg.append(ds)
                        # fused corner-combine + transpose on PE:
                        # ps[ct][c, c4, p] = sum_j g[p, c4, j*256+ct*128+c] * w_j[p]
                        for ct in range(2):
                            ps = ps_s.tile([128, 4, 128], F32, tag="ps",
                                           name="ps")
                            for c4 in range(4):
                                for j in range(4):
                                    nc.tensor.matmul(
                                        ps[:, c4, :],
                                        lhsT=g[:, c4, j * C + ct * 128:
                                               j * C + ct * 128 + 128],
                                        rhs=dg[c4][j][:],
                                        start=(j == 0),
                                        stop=(j == 3),
                                    )
                            st = sampp.tile([128, 4, 128], BF16, tag="samp",
                                            name="samp")
                            nc.scalar.activation(
                                out=st[:].rearrange("p a b -> p (a b)"),
                                in_=ps[:].rearrange("p a b -> p (a b)"),
                                func=AF.Copy)
                            # feed into both output-chunk accumulators
                            for m in range(2):
                                nc.tensor.matmul(
                                    po[m][:],
                                    lhsT=w2_sb[k * 2 + ct][:, m * 128:
                                                           (m + 1) * 128],
                                    rhs=st[:].rearrange("p a b -> p (a b)"),
                                    start=(k == 0 and ct == 0),
                                    stop=(k == K2 - 1 and ct == 1),
                                )

                    # ---- drain accumulators (one DMA per chunk) ----
                    osb = outp.tile([128, 2, CQP], BF16, tag="osb", name="osb")
                    for m in range(2):
                        nc.scalar.activation(out=osb[:, m, :], in_=po[m][:],
                                             func=AF.Copy)
                    nc.sync.dma_start(
                        out=out_t[:].rearrange("(m p) e -> p m e", m=2)
                        [:, :, cq * CQP:(cq + 1) * CQP],
                        in_=osb[:],
                    )

            # software pipeline: chunk 0's head runs standalone; each later
            # head's 9 stages interleave into the previous body's 9 taps so
            # the in-order engine queues never park on a head chain
            for rep in range(reps):
                stages0, state0 = head_stages(0)
                for stg in stages0:
                    stg()
                prev = state0
                for cq in range(NCQ):
                    if cq + 1 < NCQ:
                        nstages, nstate = head_stages(cq + 1)
                    else:
                        nstages, nstate = None, None
                    body(cq, prev["cw"], prev["w128"], nstages)
                    prev = nstate

    nc.compile()
    return nc


def host_inputs(x, w_off, b_off, w_conv):
    """Build the 8 per-core input maps (numpy only, layout prep)."""
    import ml_dtypes
    x = np.asarray(x, np.float32)
    w_off = np.asarray(w_off, np.float32)
    b_off = np.asarray(b_off, np.float32)
    w_conv = np.asarray(w_conv, np.float32)

    wof = np.ascontiguousarray(
        w_off.reshape(27, C, K2).transpose(1, 2, 0)).reshape(
            C, K2 * 27).astype(ml_dtypes.bfloat16)
    w2 = np.ascontiguousarray(
        w_conv.reshape(O, C, K2).transpose(2, 1, 0)).reshape(
            K2 * C, O).astype(ml_dtypes.bfloat16)
    # mask-logit bias row (offset bias folds into byt/bxt)
    bofm = np.concatenate([np.zeros(18, np.float32), b_off[18:27]]).reshape(
        1, 27).astype(ml_dtypes.bfloat16)
    idnb = np.eye(128, dtype=ml_dtypes.bfloat16)
    i16rep = np.tile(np.eye(16, dtype=np.float32), (1, 8)).reshape(16, 128)
    idn128f = np.eye(128, dtype=np.float32)
    idn36f = np.eye(36, dtype=np.float32)

    xp = np.zeros((B, C, 66, 66), np.float32)
    xp[:, :, 1:65, 1:65] = x

    # bf16 4-corner gather tables, one per batch image (67x67 build pad)
    xq_b = []
    for b in range(B):
        q67 = np.zeros((67, 67, C), ml_dtypes.bfloat16)
        q67[1:65, 1:65] = x[b].transpose(1, 2, 0)
        xq = np.concatenate(
            [q67[:66, :66, None], q67[:66, 1:67, None],
             q67[1:67, :66, None], q67[1:67, 1:67, None]],
            axis=2).reshape(TROWS, 4 * C)
        xq_b.append(np.ascontiguousarray(xq))

    in_maps = []
    p = np.arange(128)
    k = np.arange(K2)
    c4 = np.arange(4)
    for core in range(8):
        b, sh = core // 2, core % 2
        xs = np.ascontiguousarray(
            xp[b, :, sh * 32: sh * 32 + 34, :]).reshape(
                C, 34 * 66).astype(ml_dtypes.bfloat16)
        # pos = c4*128 + p within a chunk; row = sh*32 + pos//64 (+ 8*cq
        # added on-device); col = pos%64
        pos = c4[None, :, None] * 128 + p[None, None, :]       # [1, c4, p]
        pos = np.broadcast_to(pos, (K2, 4, 128))
        row = sh * 32 + pos // 64
        colw = pos % 64
        # padded-grid base incl. +1 pad offset and offset-conv bias
        byt = (row + (k[:, None, None] // 3)).astype(np.float32) \
            + b_off[0:18:2][:, None, None]
        bxt = (colw + (k[:, None, None] % 3)).astype(np.float32) \
            + b_off[1:18:2][:, None, None]
        byt = byt.transpose(2, 0, 1).reshape(128, 36)
        bxt = bxt.transpose(2, 0, 1).reshape(128, 36)
        bytx = np.concatenate([byt, bxt], axis=1)
        in_maps.append({
            "xs": xs, "wof": wof, "bofm": bofm, "w2": w2, "xq": xq_b[b],
            "bytx": np.ascontiguousarray(bytx),
            "idnb": idnb, "idn128f": idn128f, "idn36f": idn36f,
            "i16rep": i16rep,
        })
    return in_maps


_NC = None


def kernel(x, w_off, b_off, w_conv):
    global _NC
    if _NC is None:
        _NC = build_program()
    in_maps = host_inputs(x, w_off, b_off, w_conv)
    res = run_bass_kernel_spmd(_NC, in_maps, core_ids=list(range(8)))
    out = np.empty((B, O, H, W), np.float32)
    for core in range(8):
        b, sh = core // 2, core % 2
        out[b, :, sh * 32:(sh + 1) * 32, :] = np.asarray(
            res.results[core]["out"]).astype(np.float32).reshape(O, 32, 64)
    return out
